# revision 30
# baseline (speedup 1.0000x reference)
"""Multi-head causal attention (B=8, S=2048, D=512, H=8) on 8 Trainium2 cores.

Sharding: data-parallel over batch — batch b -> core b. No collectives.

Per-core dataflow (no on-chip transposes anywhere):
  - host supplies qT/kT/vT = q[b].T etc. in [D_MODEL, S] layout (bf16)
  - QT = Wq^T @ qT, KT = Wk^T @ kT in [D_EMBED, S] layout (bf16)
  - V packed as Vx[S, H, HD+1] bf16 with a ones-column per head, which
    makes the P@V matmul also produce the softmax denominator
  - scores computed transposed, ST_h[k, q] = K_h @ Q_h^T: for each k-tile
    one [128, 2, 512] PSUM tile holds BOTH heads of an embedding pair via
    two K=64 matmuls packed into disjoint PE row-groups (concurrent);
    causal structure: fully-masked tiles skipped, diagonal tiles emitted
    over their valid column range only
  - P^T = exp(ST/sqrt(D_EMBED)) on ACT, one instruction per k-tile
    covering both heads (3D access pattern), bf16 out; no max-subtraction
    (logits are O(1) by construction); partial tiles multiplied by 0/1
    mask tiles on GPSIMD
  - CT_h[hd+1, q] = [V_h | 1]^T @ P^T accumulated in PSUM; two concurrent
    e-streams double-buffer the accumulators so the PE never idles on one
    stream's exp
  - division by the denominator: accumulators evacuated to SBUF (DVE),
    denominator rows packed by SBUF->SBUF DMA (reaches any partition),
    ONE batched DVE reciprocal per q block, broadcast via DRAM-bounced
    stride-0 DMA, multiplied on DVE into CT = ctx^T (f32r)
  - out = CT^T @ Wo + bo in natural layout; each q block's output
    projection is deferred and interleaved into the NEXT q block's
    attention so the static PE stream never stalls on the division chain
"""

import numpy as np
import ml_dtypes

import concourse.bass as bass
import concourse.mybir as mybir
import concourse.tile as tile
from concourse import bacc

f32 = mybir.dt.float32
f32r = mybir.dt.float32r
bf16 = mybir.dt.bfloat16

B, S, DM, DE, H = 8, 2048, 512, 512, 8
HD = DE // H            # 64
HD1 = HD + 1            # 65 (V columns + ones column)
P = 128                 # partitions / k-tile
QB = 512                # q block (PSUM bank width in f32)
NQB = S // QB           # 4
NKT = S // P            # 16
NET = DE // P           # 4 embd tiles (each holds 2 heads)
NDT = DM // P           # 4 dmodel tiles
NST = S // P            # 16 seq tiles
SCALE = 1.0 / float(np.sqrt(DE))
N_CORES = 8

EXP = mybir.ActivationFunctionType.Exp



def _analyze_mask(mask):
    """Classify each [128k x 512q] S^T tile against the (head-independent)
    mask. Returns (structure, mtiles) where structure[(qb, kt)] is
    'plain' | None(skip) | (r0, uidx) and mtiles is [NU, 128, 512] f32
    multiplicative 0/1 patterns in S^T (k-row, q-col) layout."""
    m2 = np.asarray(mask, dtype=np.float32).reshape(S, S) != 0  # [q, k] True=blocked
    structure = {}
    patterns = []
    pat_index = {}
    for qb in range(NQB):
        any_r0 = []
        for kt in range(NKT):
            sub = m2[qb * QB:(qb + 1) * QB, kt * P:(kt + 1) * P]   # [q, k]
            valid = (~sub).T                                        # [k(128), q(512)]
            if not valid.any():
                structure[(qb, kt)] = None
                continue
            if valid.all():
                structure[(qb, kt)] = "plain"
                any_r0.append(0)
                continue
            cols = np.flatnonzero(valid.any(axis=0))
            c0 = int(cols[0])
            r0 = c0
            key = valid.tobytes()
            if key not in pat_index:
                pat_index[key] = len(patterns)
                patterns.append(valid.astype(np.float32))
            structure[(qb, kt)] = (r0, pat_index[key])
            any_r0.append(r0)
        assert any_r0, f"q-block {qb} has no unmasked keys"
        assert min(any_r0) == 0 or any(
            structure[(qb, kt)] == "plain" for kt in range(NKT)
        ), f"q-block {qb} has no full-width first tile"
        # every q column must have at least one valid key somewhere
        covered = np.zeros(QB, dtype=bool)
        for kt in range(NKT):
            st = structure[(qb, kt)]
            if st is None:
                continue
            sub = m2[qb * QB:(qb + 1) * QB, kt * P:(kt + 1) * P]
            covered |= (~sub).any(axis=1)
        assert covered.all(), f"q-block {qb} has fully-masked queries"
    if not patterns:
        patterns.append(np.ones((P, QB), dtype=np.float32))
    mtiles = np.stack(patterns, axis=0)
    return structure, mtiles


def _structure_key(structure, nu):
    return (tuple(sorted((k, v) for k, v in structure.items())), nu)


_BUILD_CACHE = {}


def _build(structure, nu, dummies=2):
    key = (_structure_key(structure, nu), dummies)
    if key in _BUILD_CACHE:
        return _BUILD_CACHE[key]

    nc = bacc.Bacc("TRN2", target_bir_lowering=False, debug=False,
                   enable_asserts=False, num_devices=N_CORES)

    qT = nc.dram_tensor("qT", [DM, S], bf16, kind="ExternalInput").ap()
    kT = nc.dram_tensor("kT", [DM, S], bf16, kind="ExternalInput").ap()
    vT = nc.dram_tensor("vT", [DM, S], bf16, kind="ExternalInput").ap()
    Wq = nc.dram_tensor("Wq", [DM, DE], bf16, kind="ExternalInput").ap()
    Wk = nc.dram_tensor("Wk", [DM, DE], bf16, kind="ExternalInput").ap()
    Wv = nc.dram_tensor("Wv", [DM, DE], bf16, kind="ExternalInput").ap()
    Wo = nc.dram_tensor("Wo", [DE, DM], f32r, kind="ExternalInput").ap()
    bq = nc.dram_tensor("bq", [DE, 1], f32, kind="ExternalInput").ap()
    bk = nc.dram_tensor("bk", [DE, 1], f32, kind="ExternalInput").ap()
    bv_b = nc.dram_tensor("bv_b", [P, DE], f32, kind="ExternalInput").ap()
    bo_b = nc.dram_tensor("bo_b", [P, DM], f32, kind="ExternalInput").ap()
    mt = nc.dram_tensor("mt", [nu, P, QB], bf16, kind="ExternalInput").ap()
    out = nc.dram_tensor("out", [S, DM], f32, kind="ExternalOutput").ap()
    dbg = nc.dram_tensor("dbg", [P, QB], f32, kind="ExternalOutput").ap()

    with tile.TileContext(nc) as tc:
        with nc.allow_low_precision(
                reason="f32r/bf16 intermediates are rounded on purpose"):
            _body(nc, tc, structure, nu, dummies,
                  qT=qT, kT=kT, vT=vT, Wq=Wq, Wk=Wk, Wv=Wv, Wo=Wo,
                  bq=bq, bk=bk, bv_b=bv_b, bo_b=bo_b, mt=mt, out=out,
                  dbg=dbg)
    nc.compile()
    _BUILD_CACHE[key] = nc
    return nc


def _body(nc, tc, structure, nu, dummies, *, qT, kT, vT, Wq, Wk, Wv, Wo,
          bq, bk, bv_b, bo_b, mt, out, dbg):
    from contextlib import ExitStack
    ctx = ExitStack()
    with ctx:
        const = ctx.enter_context(tc.tile_pool(name="const", bufs=1))
        res = ctx.enter_context(tc.tile_pool(name="res", bufs=1))
        ldp = ctx.enter_context(tc.tile_pool(name="ld", bufs=3))
        ppool = ctx.enter_context(tc.tile_pool(name="pp", bufs=4))
        rpool = ctx.enter_context(tc.tile_pool(name="rp", bufs=2))
        cpool = ctx.enter_context(tc.tile_pool(name="cp", bufs=5))
        dpool = ctx.enter_context(tc.tile_pool(name="dp", bufs=2))
        bpool = ctx.enter_context(tc.tile_pool(name="bp", bufs=2))
        opool = ctx.enter_context(tc.tile_pool(name="op", bufs=3))
        drp = ctx.enter_context(tc.tile_pool(name="drp", bufs=2, space="DRAM"))
        psc = ctx.enter_context(tc.tile_pool(name="psc", bufs=2, space="PSUM"))
        psct = ctx.enter_context(tc.tile_pool(name="psct", bufs=2, space="PSUM"))

        # ---- constants ----
        w_sb = {}
        for name, ap, wdt in (("Wq", Wq, bf16), ("Wk", Wk, bf16),
                              ("Wv", Wv, bf16), ("Wo", Wo, f32r)):
            t = const.tile([P, NDT, DE], wdt, tag=name)
            nc.sync.dma_start(out=t, in_=ap.rearrange("(t p) n -> p t n", p=P))
            w_sb[name] = t
        bq_sb = const.tile([P, NET, 1], f32, tag="bq")
        nc.sync.dma_start(out=bq_sb, in_=bq.rearrange("(t p) o -> p t o", p=P))
        bk_sb = const.tile([P, NET, 1], f32, tag="bk")
        nc.sync.dma_start(out=bk_sb, in_=bk.rearrange("(t p) o -> p t o", p=P))
        bv_sb = const.tile([P, DE], f32, tag="bv")
        nc.sync.dma_start(out=bv_sb, in_=bv_b)
        bo_sb = const.tile([P, DM], f32, tag="bo")
        nc.sync.dma_start(out=bo_sb, in_=bo_b)
        mt_sb = const.tile([P, nu, QB], bf16, tag="mt")
        nc.sync.dma_start(out=mt_sb, in_=mt.rearrange("u p n -> p u n"))

        # ---- resident tensors ----
        QTs = [res.tile([P, S], bf16, tag=f"qt{e}", name=f"qt{e}")
               for e in range(NET)]
        KTs = [res.tile([P, S], bf16, tag=f"kt{e}", name=f"kt{e}")
               for e in range(NET)]
        Vx = [res.tile([P, H, HD1], bf16, tag=f"vx{s}", name=f"vx{s}")
              for s in range(NST)]
        CT = [res.tile([P, S], f32r, tag=f"ct{e}", name=f"ctall{e}")
              for e in range(NET)]

        # ---- stage 1: projections ----
        for b in range(NQB):
            sl = slice(b * QB, (b + 1) * QB)
            for src, W, bias, dst in (
                (qT, "Wq", bq_sb, QTs),
                (kT, "Wk", bk_sb, KTs),
            ):
                blk = ldp.tile([P, NDT, QB], bf16, tag="ld")
                nc.sync.dma_start(
                    out=blk, in_=src.rearrange("(t p) s -> p t s", p=P)[:, :, sl])
                for ep in range(NET // 2):
                    ps = psc.tile([P, 2 * QB], f32, tag="spair", name="sp_p")
                    for j in range(2):
                        e = 2 * ep + j
                        for dmt in range(NDT):
                            nc.tensor.matmul(
                                ps[:, j * QB:(j + 1) * QB],
                                w_sb[W][:, dmt, e * P:(e + 1) * P], blk[:, dmt, :],
                                start=(dmt == 0), stop=(dmt == NDT - 1))
                    for j in range(2):
                        e = 2 * ep + j
                        nc.vector.tensor_scalar_add(
                            out=dst[e][:, sl], in0=ps[:, j * QB:(j + 1) * QB],
                            scalar1=bias[:, e, :])
            # V projection: natural layout, packed into Vx with ones column
            blk = ldp.tile([P, NDT, QB], bf16, tag="ld")
            nc.sync.dma_start(
                out=blk, in_=vT.rearrange("(t p) s -> p t s", p=P)[:, :, sl])
            for stp in range(2):
                ps = psc.tile([P, 2 * QB], f32, tag="spair", name="sp_v")
                for j in range(2):
                    st = 2 * stp + j
                    for dmt in range(NDT):
                        nc.tensor.matmul(
                            ps[:, j * QB:(j + 1) * QB],
                            blk[:, dmt, st * P:(st + 1) * P], w_sb["Wv"][:, dmt, :],
                            start=(dmt == 0), stop=(dmt == NDT - 1))
                for j in range(2):
                    s_idx = b * (QB // P) + 2 * stp + j
                    nc.vector.memset(Vx[s_idx][:, :, HD:HD1], 1.0)
                    nc.vector.tensor_add(
                        out=Vx[s_idx][:, :, 0:HD],
                        in0=ps[:, j * QB:(j + 1) * QB].rearrange(
                            "p (h d) -> p h d", h=H),
                        in1=bv_sb.rearrange("p (h d) -> p h d", h=H))

        wo_queue = []

        def emit_wo(s_idx):
            ps = psct.tile([P, DM], f32, tag=f"ct{s_idx % 2}", name="wo_ps")
            for e in range(NET):
                nc.tensor.matmul(
                    ps, CT[e][:, s_idx * P:(s_idx + 1) * P],
                    w_sb["Wo"][:, e, :],
                    start=(e == 0), stop=(e == NET - 1))
            ot = opool.tile([P, DM], f32, tag="ot")
            nc.vector.tensor_add(out=ot, in0=ps, in1=bo_sb)
            nc.sync.dma_start(out=out[s_idx * P:(s_idx + 1) * P, :], in_=ot)

        # ---- stage 2 + 3 interleaved over q blocks ----
        for qb in range(NQB):
            qsl = slice(qb * QB, (qb + 1) * QB)
            plains = [kt for kt in range(NKT) if structure[(qb, kt)] == "plain"]
            partials = sorted(
                (structure[(qb, kt)][0], structure[(qb, kt)][1], kt)
                for kt in range(NKT)
                if structure[(qb, kt)] not in (None, "plain"))
            kts = ([(kt, 0, None) for kt in plains]
                   + [(kt, r0, uidx) for r0, uidx, kt in partials])
            n_units = len(kts)

            dall = dpool.tile([2 * NET, QB], f32, tag="dall", name="dall")
            ctsbs = {}
            # two concurrent e-streams: PSUM holds one score tile and two
            # accumulators per stream, so while one stream waits on its exp
            # the PE works the other stream
            for ep in range(NET // 2):
                es = (2 * ep, 2 * ep + 1)
                cts = {}
                for e in es:
                    for hh in range(2):
                        cts[(e, hh)] = psct.tile(
                            [P, QB], f32, tag=f"ct{hh}", name=f"ct{hh}")
                firsts = {e: True for e in es}
                for ui, (kt, r0, uidx) in enumerate(kts):
                    last = ui == n_units - 1
                    for e in es:
                        sp = psc.tile([P, 2, QB], f32, tag="spair", name="sp")
                        for hh in range(2):
                            hsl = slice(hh * HD, (hh + 1) * HD)
                            nc.tensor.matmul(
                                sp[:, hh, r0:],
                                KTs[e][hsl, kt * P:(kt + 1) * P],
                                QTs[e][hsl, qb * QB + r0:(qb + 1) * QB],
                                start=True, stop=True)
                        pp = ppool.tile([P, 2, QB], bf16, tag="pp", name="pp")
                        nc.scalar.activation(out=pp[:, :, r0:],
                                             in_=sp[:, :, r0:],
                                             func=EXP, scale=SCALE)
                        if uidx is not None:
                            for hh in range(2):
                                nc.gpsimd.tensor_mul(
                                    pp[:, hh, r0:], pp[:, hh, r0:],
                                    mt_sb[:, uidx, r0:])
                        for hh in range(2):
                            h = 2 * e + hh
                            nc.tensor.matmul(
                                cts[(e, hh)][0:HD1, r0:],
                                Vx[kt][:, h, :],
                                pp[:, hh, r0:],
                                start=firsts[e], stop=last)
                        firsts[e] = False

                for e in es:
                    for hh in range(2):
                        idx = 2 * e + hh
                        ctsb = cpool.tile([HD1, QB], f32, tag=f"cs{hh}",
                                          name=f"cs{hh}")
                        nc.vector.tensor_copy(ctsb, cts[(e, hh)][0:HD1, :])
                        nc.sync.dma_start(out=dall[idx:idx + 1, :],
                                          in_=ctsb[HD:HD1, :])
                        ctsbs[(e, hh)] = ctsb
                    if wo_queue:
                        emit_wo(wo_queue.pop(0))

            # batched softmax division: ONE exact DVE reciprocal covers all
            # 8 denominator rows (cost is free-size-driven, independent of
            # partition count); per-head broadcast via DRAM-bounced step-0 DMA
            rall = rpool.tile([2 * NET, QB], f32, tag="rall", name="rall")
            nc.vector.reciprocal(out=rall, in_=dall)
            dsc = drp.tile([2 * NET, QB], f32, tag="dsc", name="dsc")
            nc.sync.dma_start(out=dsc, in_=rall)
            for e in range(NET):
                for hh in range(2):
                    idx = 2 * e + hh
                    bc = bpool.tile([HD, QB], f32, tag=f"bc{hh}", name=f"bc{hh}")
                    nc.sync.dma_start(
                        out=bc, in_=dsc[idx:idx + 1, :].to_broadcast((HD, QB)))
                    nc.vector.tensor_mul(
                        out=CT[e][hh * HD:(hh + 1) * HD, qsl],
                        in0=ctsbs[(e, hh)][0:HD, :], in1=bc)

            # queue stage 3 for this q block; emitted interleaved into the
            # NEXT q block's attention so the PE stream never stalls on the
            # division chain
            for st in range(QB // P):
                wo_queue.append(qb * (QB // P) + st)

        while wo_queue:
            emit_wo(wo_queue.pop(0))



def _make_in_maps(inputs):
    q = np.asarray(inputs["q"], dtype=np.float32)
    k = np.asarray(inputs["k"], dtype=np.float32)
    v = np.asarray(inputs["v"], dtype=np.float32)
    mask = np.asarray(inputs["mask"], dtype=np.float32)
    Wq = np.ascontiguousarray(np.asarray(inputs["Wq"], dtype=np.float32))
    Wk = np.ascontiguousarray(np.asarray(inputs["Wk"], dtype=np.float32))
    Wv = np.ascontiguousarray(np.asarray(inputs["Wv"], dtype=np.float32))
    Wo = np.ascontiguousarray(np.asarray(inputs["Wo"], dtype=np.float32))
    bq = np.asarray(inputs["bq"], dtype=np.float32).reshape(DE, 1)
    bk = np.asarray(inputs["bk"], dtype=np.float32).reshape(DE, 1)
    bv = np.asarray(inputs["bv"], dtype=np.float32)
    bo = np.asarray(inputs["bo"], dtype=np.float32)

    structure, mtiles = _analyze_mask(mask)
    nu = mtiles.shape[0]
    mtiles_bf = np.ascontiguousarray(mtiles.astype(ml_dtypes.bfloat16))
    bv_b = np.ascontiguousarray(np.broadcast_to(bv[None, :], (P, DE)).astype(np.float32))
    bo_b = np.ascontiguousarray(np.broadcast_to(bo[None, :], (P, DM)).astype(np.float32))

    qT = np.ascontiguousarray(np.transpose(q, (0, 2, 1)).astype(ml_dtypes.bfloat16))
    kT = np.ascontiguousarray(np.transpose(k, (0, 2, 1)).astype(ml_dtypes.bfloat16))
    vT = np.ascontiguousarray(np.transpose(v, (0, 2, 1)).astype(ml_dtypes.bfloat16))
    Wq = Wq.astype(ml_dtypes.bfloat16)
    Wk = Wk.astype(ml_dtypes.bfloat16)
    Wv = Wv.astype(ml_dtypes.bfloat16)

    in_maps = []
    for i in range(N_CORES):
        in_maps.append({
            "qT": qT[i], "kT": kT[i], "vT": vT[i],
            "Wq": Wq, "Wk": Wk, "Wv": Wv, "Wo": Wo,
            "bq": bq, "bk": bk, "bv_b": bv_b, "bo_b": bo_b,
            "mt": mtiles_bf,
        })
    return structure, nu, in_maps


LAST_RESULT = None


def run(inputs, trace=False):
    global LAST_RESULT
    structure, nu, in_maps = _make_in_maps(inputs)
    import os as _os2
    nc = _build(structure, nu, dummies=int(_os2.environ.get("DUMMIES", "2")))
    from concourse.bass_utils import run_bass_kernel_spmd
    res = run_bass_kernel_spmd(nc, in_maps, core_ids=list(range(N_CORES)),
                               trace=trace)
    LAST_RESULT = res
    return np.stack([r["out"] for r in res.results], axis=0)


def kernel(**inputs) -> np.ndarray:
    return run(inputs, trace=False)


# revision 31
# speedup vs baseline: 1.0130x; 1.0130x over previous
"""Multi-head causal attention (B=8, S=2048, D=512, H=8) on 8 Trainium2 cores.

Sharding: data-parallel over batch — batch b -> core b. No collectives.

Per-core dataflow (no on-chip transposes anywhere):
  - host supplies qT/kT/vT = q[b].T etc. in [D_MODEL, S] layout (bf16)
  - QT = Wq^T @ qT, KT = Wk^T @ kT in [D_EMBED, S] layout (bf16)
  - V packed as Vx[S, H, HD+1] bf16 with a ones-column per head, which
    makes the P@V matmul also produce the softmax denominator
  - scores computed transposed, ST_h[k, q] = K_h @ Q_h^T: for each k-tile
    one [128, 2, 512] PSUM tile holds BOTH heads of an embedding pair via
    two K=64 matmuls packed into disjoint PE row-groups (concurrent);
    causal structure: fully-masked tiles skipped, diagonal tiles emitted
    over their valid column range only
  - P^T = exp(ST/sqrt(D_EMBED)) on ACT, one instruction per k-tile
    covering both heads (3D access pattern), bf16 out; no max-subtraction
    (logits are O(1) by construction); partial tiles multiplied by 0/1
    mask tiles on GPSIMD
  - CT_h[hd+1, q] = [V_h | 1]^T @ P^T accumulated in PSUM; two concurrent
    e-streams double-buffer the accumulators so the PE never idles on one
    stream's exp
  - division by the denominator: accumulators evacuated to SBUF (DVE),
    denominator rows packed by SBUF->SBUF DMA (reaches any partition),
    ONE batched DVE reciprocal per q block, broadcast via DRAM-bounced
    stride-0 DMA, multiplied on DVE into CT = ctx^T (f32r)
  - out = CT^T @ Wo + bo in natural layout; each q block's output
    projection is deferred and interleaved into the NEXT q block's
    attention so the static PE stream never stalls on the division chain
"""

import numpy as np
import ml_dtypes

import concourse.bass as bass
import concourse.mybir as mybir
import concourse.tile as tile
from concourse import bacc

f32 = mybir.dt.float32
f32r = mybir.dt.float32r
bf16 = mybir.dt.bfloat16

B, S, DM, DE, H = 8, 2048, 512, 512, 8
HD = DE // H            # 64
HD1 = HD + 1            # 65 (V columns + ones column)
P = 128                 # partitions / k-tile
QB = 512                # q block (PSUM bank width in f32)
NQB = S // QB           # 4
NKT = S // P            # 16
NET = DE // P           # 4 embd tiles (each holds 2 heads)
NDT = DM // P           # 4 dmodel tiles
NST = S // P            # 16 seq tiles
SCALE = 1.0 / float(np.sqrt(DE))
N_CORES = 8

EXP = mybir.ActivationFunctionType.Exp



def _analyze_mask(mask):
    """Classify each [128k x 512q] S^T tile against the (head-independent)
    mask. Returns (structure, mtiles) where structure[(qb, kt)] is
    'plain' | None(skip) | (r0, uidx) and mtiles is [NU, 128, 512] f32
    multiplicative 0/1 patterns in S^T (k-row, q-col) layout."""
    m2 = np.asarray(mask, dtype=np.float32).reshape(S, S) != 0  # [q, k] True=blocked
    structure = {}
    patterns = []
    pat_index = {}
    for qb in range(NQB):
        any_r0 = []
        for kt in range(NKT):
            sub = m2[qb * QB:(qb + 1) * QB, kt * P:(kt + 1) * P]   # [q, k]
            valid = (~sub).T                                        # [k(128), q(512)]
            if not valid.any():
                structure[(qb, kt)] = None
                continue
            if valid.all():
                structure[(qb, kt)] = "plain"
                any_r0.append(0)
                continue
            cols = np.flatnonzero(valid.any(axis=0))
            c0 = int(cols[0])
            r0 = c0
            key = valid.tobytes()
            if key not in pat_index:
                pat_index[key] = len(patterns)
                patterns.append(valid.astype(np.float32))
            structure[(qb, kt)] = (r0, pat_index[key])
            any_r0.append(r0)
        assert any_r0, f"q-block {qb} has no unmasked keys"
        assert min(any_r0) == 0 or any(
            structure[(qb, kt)] == "plain" for kt in range(NKT)
        ), f"q-block {qb} has no full-width first tile"
        # every q column must have at least one valid key somewhere
        covered = np.zeros(QB, dtype=bool)
        for kt in range(NKT):
            st = structure[(qb, kt)]
            if st is None:
                continue
            sub = m2[qb * QB:(qb + 1) * QB, kt * P:(kt + 1) * P]
            covered |= (~sub).any(axis=1)
        assert covered.all(), f"q-block {qb} has fully-masked queries"
    if not patterns:
        patterns.append(np.ones((P, QB), dtype=np.float32))
    mtiles = np.stack(patterns, axis=0)
    return structure, mtiles


def _structure_key(structure, nu):
    return (tuple(sorted((k, v) for k, v in structure.items())), nu)


_BUILD_CACHE = {}


def _build(structure, nu, dummies=2):
    key = (_structure_key(structure, nu), dummies)
    if key in _BUILD_CACHE:
        return _BUILD_CACHE[key]

    nc = bacc.Bacc("TRN2", target_bir_lowering=False, debug=False,
                   enable_asserts=False, num_devices=N_CORES)

    qT = nc.dram_tensor("qT", [DM, S], bf16, kind="ExternalInput").ap()
    kT = nc.dram_tensor("kT", [DM, S], bf16, kind="ExternalInput").ap()
    vT = nc.dram_tensor("vT", [DM, S], bf16, kind="ExternalInput").ap()
    Wq = nc.dram_tensor("Wq", [DM, DE], bf16, kind="ExternalInput").ap()
    Wk = nc.dram_tensor("Wk", [DM, DE], bf16, kind="ExternalInput").ap()
    Wv = nc.dram_tensor("Wv", [DM, DE], bf16, kind="ExternalInput").ap()
    Wo = nc.dram_tensor("Wo", [DE, DM], bf16, kind="ExternalInput").ap()
    bq = nc.dram_tensor("bq", [DE, 1], f32, kind="ExternalInput").ap()
    bk = nc.dram_tensor("bk", [DE, 1], f32, kind="ExternalInput").ap()
    bv_b = nc.dram_tensor("bv_b", [P, DE], f32, kind="ExternalInput").ap()
    bo_b = nc.dram_tensor("bo_b", [P, DM], f32, kind="ExternalInput").ap()
    mt = nc.dram_tensor("mt", [nu, P, QB], bf16, kind="ExternalInput").ap()
    out = nc.dram_tensor("out", [S, DM], f32, kind="ExternalOutput").ap()
    dbg = nc.dram_tensor("dbg", [P, QB], f32, kind="ExternalOutput").ap()

    with tile.TileContext(nc) as tc:
        with nc.allow_low_precision(
                reason="f32r/bf16 intermediates are rounded on purpose"):
            _body(nc, tc, structure, nu, dummies,
                  qT=qT, kT=kT, vT=vT, Wq=Wq, Wk=Wk, Wv=Wv, Wo=Wo,
                  bq=bq, bk=bk, bv_b=bv_b, bo_b=bo_b, mt=mt, out=out,
                  dbg=dbg)
    nc.compile()
    _BUILD_CACHE[key] = nc
    return nc


def _body(nc, tc, structure, nu, dummies, *, qT, kT, vT, Wq, Wk, Wv, Wo,
          bq, bk, bv_b, bo_b, mt, out, dbg):
    from contextlib import ExitStack
    ctx = ExitStack()
    with ctx:
        const = ctx.enter_context(tc.tile_pool(name="const", bufs=1))
        res = ctx.enter_context(tc.tile_pool(name="res", bufs=1))
        ldp = ctx.enter_context(tc.tile_pool(name="ld", bufs=4))
        ppool = ctx.enter_context(tc.tile_pool(name="pp", bufs=8))
        rpool = ctx.enter_context(tc.tile_pool(name="rp", bufs=2))
        cpool = ctx.enter_context(tc.tile_pool(name="cp", bufs=5))
        dpool = ctx.enter_context(tc.tile_pool(name="dp", bufs=2))
        bpool = ctx.enter_context(tc.tile_pool(name="bp", bufs=2))
        opool = ctx.enter_context(tc.tile_pool(name="op", bufs=4))
        drp = ctx.enter_context(tc.tile_pool(name="drp", bufs=2, space="DRAM"))
        psc = ctx.enter_context(tc.tile_pool(name="psc", bufs=2, space="PSUM"))
        psct = ctx.enter_context(tc.tile_pool(name="psct", bufs=2, space="PSUM"))

        # ---- constants ----
        w_sb = {}
        for name, ap, wdt in (("Wq", Wq, bf16), ("Wk", Wk, bf16),
                              ("Wv", Wv, bf16), ("Wo", Wo, bf16)):
            t = const.tile([P, NDT, DE], wdt, tag=name)
            nc.sync.dma_start(out=t, in_=ap.rearrange("(t p) n -> p t n", p=P))
            w_sb[name] = t
        bq_sb = const.tile([P, NET, 1], f32, tag="bq")
        nc.sync.dma_start(out=bq_sb, in_=bq.rearrange("(t p) o -> p t o", p=P))
        bk_sb = const.tile([P, NET, 1], f32, tag="bk")
        nc.sync.dma_start(out=bk_sb, in_=bk.rearrange("(t p) o -> p t o", p=P))
        bv_sb = const.tile([P, DE], f32, tag="bv")
        nc.sync.dma_start(out=bv_sb, in_=bv_b)
        bo_sb = const.tile([P, DM], f32, tag="bo")
        nc.sync.dma_start(out=bo_sb, in_=bo_b)
        mt_sb = const.tile([P, nu, QB], bf16, tag="mt")
        nc.sync.dma_start(out=mt_sb, in_=mt.rearrange("u p n -> p u n"))

        # ---- resident tensors ----
        QTs = [res.tile([P, S], bf16, tag=f"qt{e}", name=f"qt{e}")
               for e in range(NET)]
        KTs = [res.tile([P, S], bf16, tag=f"kt{e}", name=f"kt{e}")
               for e in range(NET)]
        Vx = [res.tile([P, H, HD1], bf16, tag=f"vx{s}", name=f"vx{s}")
              for s in range(NST)]
        CT = [res.tile([P, S], bf16, tag=f"ct{e}", name=f"ctall{e}")
              for e in range(NET)]

        # ---- stage 1: projections ----
        for b in range(NQB):
            sl = slice(b * QB, (b + 1) * QB)
            for src, W, bias, dst in (
                (qT, "Wq", bq_sb, QTs),
                (kT, "Wk", bk_sb, KTs),
            ):
                blk = ldp.tile([P, NDT, QB], bf16, tag="ld")
                nc.sync.dma_start(
                    out=blk, in_=src.rearrange("(t p) s -> p t s", p=P)[:, :, sl])
                for ep in range(NET // 2):
                    ps = psc.tile([P, 2 * QB], f32, tag="spair", name="sp_p")
                    for j in range(2):
                        e = 2 * ep + j
                        for dmt in range(NDT):
                            nc.tensor.matmul(
                                ps[:, j * QB:(j + 1) * QB],
                                w_sb[W][:, dmt, e * P:(e + 1) * P], blk[:, dmt, :],
                                start=(dmt == 0), stop=(dmt == NDT - 1))
                    for j in range(2):
                        e = 2 * ep + j
                        nc.vector.tensor_scalar_add(
                            out=dst[e][:, sl], in0=ps[:, j * QB:(j + 1) * QB],
                            scalar1=bias[:, e, :])
            # V projection: natural layout, packed into Vx with ones column
            blk = ldp.tile([P, NDT, QB], bf16, tag="ld")
            nc.sync.dma_start(
                out=blk, in_=vT.rearrange("(t p) s -> p t s", p=P)[:, :, sl])
            for stp in range(2):
                ps = psc.tile([P, 2 * QB], f32, tag="spair", name="sp_v")
                for j in range(2):
                    st = 2 * stp + j
                    for dmt in range(NDT):
                        nc.tensor.matmul(
                            ps[:, j * QB:(j + 1) * QB],
                            blk[:, dmt, st * P:(st + 1) * P], w_sb["Wv"][:, dmt, :],
                            start=(dmt == 0), stop=(dmt == NDT - 1))
                for j in range(2):
                    s_idx = b * (QB // P) + 2 * stp + j
                    nc.vector.memset(Vx[s_idx][:, :, HD:HD1], 1.0)
                    nc.vector.tensor_add(
                        out=Vx[s_idx][:, :, 0:HD],
                        in0=ps[:, j * QB:(j + 1) * QB].rearrange(
                            "p (h d) -> p h d", h=H),
                        in1=bv_sb.rearrange("p (h d) -> p h d", h=H))

        wo_queue = []

        def emit_wo(s_idx):
            ps = psct.tile([P, DM], f32, tag=f"ct{s_idx % 2}", name="wo_ps")
            for e in range(NET):
                nc.tensor.matmul(
                    ps, CT[e][:, s_idx * P:(s_idx + 1) * P],
                    w_sb["Wo"][:, e, :],
                    start=(e == 0), stop=(e == NET - 1))
            ot = opool.tile([P, DM], f32, tag="ot")
            nc.vector.tensor_add(out=ot, in0=ps, in1=bo_sb)
            nc.sync.dma_start(out=out[s_idx * P:(s_idx + 1) * P, :], in_=ot)

        # ---- stage 2 + 3 interleaved over q blocks ----
        for qb in range(NQB):
            qsl = slice(qb * QB, (qb + 1) * QB)
            plains = [kt for kt in range(NKT) if structure[(qb, kt)] == "plain"]
            partials = sorted(
                (structure[(qb, kt)][0], structure[(qb, kt)][1], kt)
                for kt in range(NKT)
                if structure[(qb, kt)] not in (None, "plain"))
            kts = ([(kt, 0, None) for kt in plains]
                   + [(kt, r0, uidx) for r0, uidx, kt in partials])
            n_units = len(kts)

            dall = dpool.tile([2 * NET, QB], f32, tag="dall", name="dall")
            ctsbs = {}
            # two concurrent e-streams: PSUM holds one score tile and two
            # accumulators per stream, so while one stream waits on its exp
            # the PE works the other stream
            for ep in range(NET // 2):
                es = (2 * ep, 2 * ep + 1)
                cts = {}
                for e in es:
                    for hh in range(2):
                        cts[(e, hh)] = psct.tile(
                            [P, QB], f32, tag=f"ct{hh}", name=f"ct{hh}")
                firsts = {e: True for e in es}
                for ui, (kt, r0, uidx) in enumerate(kts):
                    last = ui == n_units - 1
                    for e in es:
                        sp = psc.tile([P, 2, QB], f32, tag="spair", name="sp")
                        for hh in range(2):
                            hsl = slice(hh * HD, (hh + 1) * HD)
                            nc.tensor.matmul(
                                sp[:, hh, r0:],
                                KTs[e][hsl, kt * P:(kt + 1) * P],
                                QTs[e][hsl, qb * QB + r0:(qb + 1) * QB],
                                start=True, stop=True)
                        pp = ppool.tile([P, 2, QB], bf16, tag="pp", name="pp")
                        nc.scalar.activation(out=pp[:, :, r0:],
                                             in_=sp[:, :, r0:],
                                             func=EXP, scale=SCALE)
                        if uidx is not None:
                            for hh in range(2):
                                nc.gpsimd.tensor_mul(
                                    pp[:, hh, r0:], pp[:, hh, r0:],
                                    mt_sb[:, uidx, r0:])
                        for hh in range(2):
                            h = 2 * e + hh
                            nc.tensor.matmul(
                                cts[(e, hh)][0:HD1, r0:],
                                Vx[kt][:, h, :],
                                pp[:, hh, r0:],
                                start=firsts[e], stop=last)
                        firsts[e] = False

                for e in es:
                    for hh in range(2):
                        idx = 2 * e + hh
                        ctsb = cpool.tile([HD1, QB], f32, tag=f"cs{hh}",
                                          name=f"cs{hh}")
                        nc.vector.tensor_copy(ctsb, cts[(e, hh)][0:HD1, :])
                        nc.sync.dma_start(out=dall[idx:idx + 1, :],
                                          in_=ctsb[HD:HD1, :])
                        ctsbs[(e, hh)] = ctsb
                    if wo_queue:
                        emit_wo(wo_queue.pop(0))

            # batched softmax division: ONE exact DVE reciprocal covers all
            # 8 denominator rows (cost is free-size-driven, independent of
            # partition count); per-head broadcast via DRAM-bounced step-0 DMA
            rall = rpool.tile([2 * NET, QB], f32, tag="rall", name="rall")
            nc.vector.reciprocal(out=rall, in_=dall)
            dsc = drp.tile([2 * NET, QB], f32, tag="dsc", name="dsc")
            nc.sync.dma_start(out=dsc, in_=rall)
            for e in range(NET):
                for hh in range(2):
                    idx = 2 * e + hh
                    bc = bpool.tile([HD, QB], f32, tag=f"bc{hh}", name=f"bc{hh}")
                    nc.sync.dma_start(
                        out=bc, in_=dsc[idx:idx + 1, :].to_broadcast((HD, QB)))
                    nc.vector.tensor_mul(
                        out=CT[e][hh * HD:(hh + 1) * HD, qsl],
                        in0=ctsbs[(e, hh)][0:HD, :], in1=bc)

            # queue stage 3 for this q block; emitted interleaved into the
            # NEXT q block's attention so the PE stream never stalls on the
            # division chain
            for st in range(QB // P):
                wo_queue.append(qb * (QB // P) + st)

        while wo_queue:
            emit_wo(wo_queue.pop(0))



def _make_in_maps(inputs):
    q = np.asarray(inputs["q"], dtype=np.float32)
    k = np.asarray(inputs["k"], dtype=np.float32)
    v = np.asarray(inputs["v"], dtype=np.float32)
    mask = np.asarray(inputs["mask"], dtype=np.float32)
    Wq = np.ascontiguousarray(np.asarray(inputs["Wq"], dtype=np.float32))
    Wk = np.ascontiguousarray(np.asarray(inputs["Wk"], dtype=np.float32))
    Wv = np.ascontiguousarray(np.asarray(inputs["Wv"], dtype=np.float32))
    Wo = np.ascontiguousarray(np.asarray(inputs["Wo"], dtype=np.float32))
    bq = np.asarray(inputs["bq"], dtype=np.float32).reshape(DE, 1)
    bk = np.asarray(inputs["bk"], dtype=np.float32).reshape(DE, 1)
    bv = np.asarray(inputs["bv"], dtype=np.float32)
    bo = np.asarray(inputs["bo"], dtype=np.float32)

    structure, mtiles = _analyze_mask(mask)
    nu = mtiles.shape[0]
    mtiles_bf = np.ascontiguousarray(mtiles.astype(ml_dtypes.bfloat16))
    bv_b = np.ascontiguousarray(np.broadcast_to(bv[None, :], (P, DE)).astype(np.float32))
    bo_b = np.ascontiguousarray(np.broadcast_to(bo[None, :], (P, DM)).astype(np.float32))

    qT = np.ascontiguousarray(np.transpose(q, (0, 2, 1)).astype(ml_dtypes.bfloat16))
    kT = np.ascontiguousarray(np.transpose(k, (0, 2, 1)).astype(ml_dtypes.bfloat16))
    vT = np.ascontiguousarray(np.transpose(v, (0, 2, 1)).astype(ml_dtypes.bfloat16))
    Wq = Wq.astype(ml_dtypes.bfloat16)
    Wk = Wk.astype(ml_dtypes.bfloat16)
    Wv = Wv.astype(ml_dtypes.bfloat16)
    Wo = Wo.astype(ml_dtypes.bfloat16)

    in_maps = []
    for i in range(N_CORES):
        in_maps.append({
            "qT": qT[i], "kT": kT[i], "vT": vT[i],
            "Wq": Wq, "Wk": Wk, "Wv": Wv, "Wo": Wo,
            "bq": bq, "bk": bk, "bv_b": bv_b, "bo_b": bo_b,
            "mt": mtiles_bf,
        })
    return structure, nu, in_maps


LAST_RESULT = None


def run(inputs, trace=False):
    global LAST_RESULT
    structure, nu, in_maps = _make_in_maps(inputs)
    import os as _os2
    nc = _build(structure, nu, dummies=int(_os2.environ.get("DUMMIES", "2")))
    from concourse.bass_utils import run_bass_kernel_spmd
    res = run_bass_kernel_spmd(nc, in_maps, core_ids=list(range(N_CORES)),
                               trace=trace)
    LAST_RESULT = res
    return np.stack([r["out"] for r in res.results], axis=0)


def kernel(**inputs) -> np.ndarray:
    return run(inputs, trace=False)
